# revision 9
# baseline (speedup 1.0000x reference)
"""Trainium2 Bass kernel for nn_LoraAttention.

Math (reference): qkv = x@W_qkv.T; lora full proj ql/vl = split(x@W_lora.T + b_lora)
(K-part discarded); low-rank dq = (x@A_q.T)@B_q.T*1/8 (same for v); softmax
attention over H=16 heads, D=64; out = attn_cat@W_out.T + b_out.

Host-side algebra folds every LoRA term into the projection weights:
  Wq_eff = W_qkv[q] + W_lora[q] + (B_q@A_q)/8      (q bias b_lora[q] kept)
  Wk_eff = W_qkv[k]                                 (no bias)
  Wv_eff = W_qkv[v] + W_lora[v] + (B_v@A_v)/8
  v bias b_lora[v] commutes through softmax -> folded into host-side output
  bias: b_eff = b_out + W_out @ b_lora[v].

Sharding: 8 cores = 4 batches x 2 head-groups (8 heads each).  Each core
projects QKV for its heads, does attention, and computes a partial output
projection over its 512 concat dims; host sums the two partials per batch.

Device dataflow per core (matmuls bf16 in / fp32 accum):
  phase 1: QKV projections (full 128x128-weight matmuls).
  phase 2: attention: S^T = K^T@Q per head via row-packed (tile_position)
    pairs of K=64 matmuls; exp on ScalarE from PSUM (scale=1/8, bf16 out);
    P@[1|V] matmuls put the softmax denominator in row 0 and raw attention
    in rows 1..64; DVE computes reciprocal rows; raw attention staged bf16.
  phase 3: K=1 ones-matmuls broadcast each reciprocal row across partitions,
    DVE multiplies raw attention by it, DMA packs into (d,128)x(nq,512) tiles.
  phase 4: output projection.
  Phases are separated by strict barriers: interleaving full-array matmul
  accumulation groups with tile_position matmuls corrupts PSUM on HW.
"""

import numpy as np
import ml_dtypes

import concourse.bacc as bacc
import concourse.tile as tile
from concourse import mybir
from concourse.bass_utils import run_bass_kernel_spmd

B, N, C = 4, 2048, 1024
H, D = 16, 64
LORA_SCALE = 1.0 / 8.0
ATTN_SCALE = float(D) ** -0.5  # 0.125

f32 = mybir.dt.float32
bf16 = mybir.dt.bfloat16
BF = ml_dtypes.bfloat16

NQ = 4           # token chunks of 512 for moving operands
MQ = 16          # key/token chunks of 128 for S^T partition dim
KC = 8           # contraction chunks of 128 over C
PAIRS = 4        # head pairs per core (8 local heads)

_cache: dict = {}
_DEBUG = False


def _build_program():
    nc = bacc.Bacc("TRN2", target_bir_lowering=False, debug=False, num_devices=8)

    xT_d = nc.dram_tensor("xT", [C, N], bf16, kind="ExternalInput").ap()
    wqk_d = nc.dram_tensor("wqk", [C, 1024], bf16, kind="ExternalInput").ap()
    wv_d = nc.dram_tensor("wv", [C, 512], bf16, kind="ExternalInput").ap()
    wo_d = nc.dram_tensor("wo", [512, C], bf16, kind="ExternalInput").ap()
    bq_d = nc.dram_tensor("bq", [128, 4], f32, kind="ExternalInput").ap()
    outT_d = nc.dram_tensor("outT", [C, N], f32, kind="ExternalOutput").ap()
    if _DEBUG:
        recall_d = nc.dram_tensor("d_recall", [32, 512], f32, kind="ExternalOutput").ap()
        araw_d = nc.dram_tensor("d_araw", [32, 64, 512], f32, kind="ExternalOutput").ap()
        acat_d = nc.dram_tensor("d_acat", [16, 128, 512], f32, kind="ExternalOutput").ap()
        rb_d = nc.dram_tensor("d_rb", [2, 64, 512], f32, kind="ExternalOutput").ap()

    EXP = mybir.ActivationFunctionType.Exp

    with tile.TileContext(nc) as tc:
        with (
            tc.tile_pool(name="win", bufs=1) as win,        # weights + x + consts
            tc.tile_pool(name="kq", bufs=1) as kqp,         # K/Q bf16 tiles
            tc.tile_pool(name="vp", bufs=1) as vp,          # [1|V] tiles
            tc.tile_pool(name="pex", bufs=4) as pex,        # exp outputs
            tc.tile_pool(name="ar", bufs=1) as arp,         # raw attn staging
            tc.tile_pool(name="acat", bufs=1) as acatp,     # normalized attn (d, nq)
            tc.tile_pool(name="scr", bufs=4) as scr,        # small scratch
            tc.tile_pool(name="osb", bufs=3) as osbp,       # out eviction
            tc.tile_pool(name="pp", bufs=2, space="PSUM") as pp,
        ):
            # ---- loads ----
            xt = []
            for kc in range(KC):
                t = win.tile([128, N], bf16, tag=f"xt{kc}")
                nc.sync.dma_start(t[:], xT_d[kc * 128:(kc + 1) * 128, :])
                xt.append(t)
            wqk = []
            for kc in range(KC):
                t = win.tile([128, 1024], bf16, tag=f"wqk{kc}")
                nc.sync.dma_start(t[:], wqk_d[kc * 128:(kc + 1) * 128, :])
                wqk.append(t)
            wv = []
            for kc in range(KC):
                t = win.tile([128, 512], bf16, tag=f"wv{kc}")
                nc.sync.dma_start(t[:], wv_d[kc * 128:(kc + 1) * 128, :])
                wv.append(t)
            wo = []
            for dc in range(4):
                t = win.tile([128, 1024], bf16, tag=f"wo{dc}")
                nc.sync.dma_start(t[:], wo_d[dc * 128:(dc + 1) * 128, :])
                wo.append(t)
            bqt = win.tile([128, 4], f32, tag="bq")
            nc.sync.dma_start(bqt[:], bq_d[:])
            ones64 = win.tile([1, 64], f32, tag="ones64")
            nc.vector.memset(ones64[:], 1.0)
            rec_all = win.tile([32, 512], f32, tag="rec_all")

            acat = [[None] * PAIRS for _ in range(NQ)]
            araw = {}

            with (
                tc.tile_pool(name="sp", bufs=2, space="PSUM") as spp,
                tc.tile_pool(name="ap", bufs=1, space="PSUM") as app,
            ):
                # ---- phase 1: projections ----
                vts = []
                for mq in range(MQ):
                    vt = vp.tile([128, 8, 65], bf16, tag=f"v{mq}")
                    nc.vector.memset(vt[:, :, 64:65], 1.0)
                    ps = pp.tile([128, 512], f32, tag="pp")
                    for kc in range(KC):
                        nc.tensor.matmul(
                            ps[:], xt[kc][:, mq * 128:(mq + 1) * 128], wv[kc][:],
                            start=(kc == 0), stop=(kc == KC - 1),
                        )
                    nc.vector.tensor_copy(
                        vt[:, :, 0:64], ps[:].rearrange("p (h e) -> p h e", h=8)
                    )
                    vts.append(vt)

                kts, qts = [], []
                for t in range(PAIRS):
                    kt = kqp.tile([128, N], bf16, tag=f"k{t}")
                    for nq in range(NQ):
                        ps = pp.tile([128, 512], f32, tag="pp")
                        for kc in range(KC):
                            nc.tensor.matmul(
                                ps[:],
                                wqk[kc][:, 512 + t * 128:512 + (t + 1) * 128],
                                xt[kc][:, nq * 512:(nq + 1) * 512],
                                start=(kc == 0), stop=(kc == KC - 1),
                            )
                        nc.vector.tensor_copy(kt[:, nq * 512:(nq + 1) * 512], ps[:])
                    qt = kqp.tile([128, N], bf16, tag=f"q{t}")
                    for nq in range(NQ):
                        ps = pp.tile([128, 512], f32, tag="pp")
                        for kc in range(KC):
                            nc.tensor.matmul(
                                ps[:],
                                wqk[kc][:, t * 128:(t + 1) * 128],
                                xt[kc][:, nq * 512:(nq + 1) * 512],
                                start=(kc == 0), stop=(kc == KC - 1),
                            )
                        nc.vector.tensor_scalar_add(
                            qt[:, nq * 512:(nq + 1) * 512], ps[:], bqt[:, t:t + 1]
                        )
                    kts.append(kt)
                    qts.append(qt)

                tc.strict_bb_all_engine_barrier()

                # ---- phase 2: attention ----
                for t in range(PAIRS):
                    kt, qt = kts[t], qts[t]
                    for nq in range(NQ):
                        atA = app.tile([65, 512], f32, tag="atA")
                        atB = app.tile([65, 512], f32, tag="atB")
                        for mq in range(MQ):
                            sp = spp.tile([128, 1024], f32, tag="sp")
                            nc.tensor.matmul(
                                sp[:, 0:512],
                                kt[0:64, mq * 128:(mq + 1) * 128],
                                qt[0:64, nq * 512:(nq + 1) * 512],
                                start=True, stop=True, tile_position=(0, 0),
                            )
                            nc.tensor.matmul(
                                sp[:, 512:1024],
                                kt[64:128, mq * 128:(mq + 1) * 128],
                                qt[64:128, nq * 512:(nq + 1) * 512],
                                start=True, stop=True, tile_position=(64, 0),
                            )
                            pe = pex.tile([128, 1024], bf16, tag="pe")
                            nc.scalar.activation(pe[:], sp[:], EXP, scale=ATTN_SCALE)
                            nc.tensor.matmul(
                                atA[:], vts[mq][:, 2 * t, :], pe[:, 0:512],
                                start=(mq == 0), stop=(mq == MQ - 1),
                            )
                            nc.tensor.matmul(
                                atB[:], vts[mq][:, 2 * t + 1, :], pe[:, 512:1024],
                                start=(mq == 0), stop=(mq == MQ - 1),
                            )
                        for at, half in ((atA, 0), (atB, 1)):
                            idx = (t * NQ + nq) * 2 + half
                            rec = scr.tile([65, 512], f32, tag="rec")
                            nc.vector.tensor_copy(rec[64:65, :], at[64:65, :])
                            nc.sync.dma_start(rec_all[idx:idx + 1, :], rec[64:65, :])
                            ar = arp.tile([64, 512], bf16, tag=f"ar{idx}")
                            nc.vector.tensor_copy(ar[:], at[0:64, :])
                            araw[idx] = ar

            tc.strict_bb_all_engine_barrier()

            # ---- phase 3: normalize (K=1 broadcast matmuls + DVE mul) ----
            with tc.tile_pool(name="rb", bufs=2, space="PSUM") as rbp:
                for t in range(PAIRS):
                    for nq in range(NQ):
                        ac = acatp.tile([128, 512], bf16, tag=f"ac{nq}_{t}")
                        acat[nq][t] = ac
                        for half in range(2):
                            idx = (t * NQ + nq) * 2 + half
                            r0 = scr.tile([1, 512], f32, tag="r0")
                            nc.sync.dma_start(r0[0:1, :], rec_all[idx:idx + 1, :])
                            rr = scr.tile([1, 512], f32, tag="rr")
                            nc.vector.reciprocal_approx_fast(rr[0:1, :], r0[0:1, :])
                            rb = rbp.tile([64, 512], f32, tag="rb")
                            nc.tensor.matmul(
                                rb[:], ones64[0:1, :], rr[0:1, :],
                                start=True, stop=True,
                            )
                            acn = scr.tile([64, 512], bf16, tag="acn")
                            nc.vector.tensor_mul(
                                acn[:], araw[idx][:], rb[:]
                            )
                            nc.sync.dma_start(
                                ac[half * 64:(half + 1) * 64, :], acn[:]
                            )
                            if _DEBUG:
                                dr = osbp.tile([128, 512], f32, tag="d_ac")
                                nc.vector.tensor_copy(dr[0:64, :], araw[idx][:])
                                nc.sync.dma_start(araw_d[idx, :, :], dr[0:64, :])
                                if idx < 2:
                                    db = osbp.tile([128, 512], f32, tag="d_ac")
                                    nc.vector.tensor_copy(db[0:64, :], rb[:])
                                    nc.sync.dma_start(rb_d[idx, :, :], db[0:64, :])

            if _DEBUG:
                nc.sync.dma_start(recall_d[:], rec_all[:])
                for nq in range(NQ):
                    for t in range(PAIRS):
                        da = osbp.tile([128, 512], f32, tag="d_ac")
                        nc.vector.tensor_copy(da[:], acat[nq][t][:])
                        nc.sync.dma_start(acat_d[nq * 4 + t, :, :], da[:])

            tc.strict_bb_all_engine_barrier()

            # ---- phase 4: output projection ----
            for nq in range(NQ):
                for cc in range(8):
                    ps = pp.tile([128, 512], f32, tag="pp")
                    for dc in range(4):
                        nc.tensor.matmul(
                            ps[:],
                            wo[dc][:, cc * 128:(cc + 1) * 128],
                            acat[nq][dc][:],
                            start=(dc == 0), stop=(dc == 3),
                        )
                    ob = osbp.tile([128, 512], f32, tag="ob")
                    nc.vector.tensor_copy(ob[:], ps[:])
                    nc.sync.dma_start(
                        outT_d[cc * 128:(cc + 1) * 128, nq * 512:(nq + 1) * 512],
                        ob[:],
                    )

    nc.compile()
    return nc


def _get_program():
    if "nc" not in _cache:
        _cache["nc"] = _build_program()
    return _cache["nc"]


def _prep_in_maps(x, W_qkv, W_lora, b_lora, A_q, B_q, A_v, B_v, W_out):
    HD = H * D  # 1024
    Wq = W_qkv[0:HD] + W_lora[0:HD] + LORA_SCALE * (B_q @ A_q)
    Wk = W_qkv[HD:2 * HD]
    Wv = W_qkv[2 * HD:3 * HD] + W_lora[2 * HD:3 * HD] + LORA_SCALE * (B_v @ A_v)
    bq = b_lora[0:HD]

    xT = [np.ascontiguousarray(x[b].T).astype(BF) for b in range(B)]
    in_maps = []
    for c in range(8):
        b, hg = divmod(c, 2)
        sel = slice(hg * 512, (hg + 1) * 512)
        wqk_c = np.ascontiguousarray(
            np.concatenate([Wq[sel], Wk[sel]], axis=0).T
        ).astype(BF)
        wv_c = np.ascontiguousarray(Wv[sel].T).astype(BF)
        wo_c = np.ascontiguousarray(W_out[:, sel].T).astype(BF)
        bq_c = np.ascontiguousarray(bq[sel].reshape(4, 128).T).astype(np.float32)
        in_maps.append({
            "xT": xT[b], "wqk": wqk_c, "wv": wv_c, "wo": wo_c, "bq": bq_c,
        })
    return in_maps


def kernel(x, W_qkv, W_lora, b_lora, A_q, B_q, A_v, B_v, W_out, b_out):
    x = np.asarray(x, np.float32)
    W_qkv = np.asarray(W_qkv, np.float32)
    W_lora = np.asarray(W_lora, np.float32)
    b_lora = np.asarray(b_lora, np.float32)
    A_q = np.asarray(A_q, np.float32)
    B_q = np.asarray(B_q, np.float32)
    A_v = np.asarray(A_v, np.float32)
    B_v = np.asarray(B_v, np.float32)
    W_out = np.asarray(W_out, np.float32)
    b_out = np.asarray(b_out, np.float32)

    in_maps = _prep_in_maps(x, W_qkv, W_lora, b_lora, A_q, B_q, A_v, B_v, W_out)
    b_eff = b_out + W_out @ b_lora[2 * H * D:3 * H * D]

    nc = _get_program()
    res = run_bass_kernel_spmd(nc, in_maps, list(range(8)))

    out = np.empty((B, N, C), np.float32)
    for b in range(B):
        acc = res.results[2 * b]["outT"] + res.results[2 * b + 1]["outT"]
        acc += b_eff[:, None]
        out[b] = acc.T
    return out


# revision 12
# speedup vs baseline: 1.0198x; 1.0198x over previous
"""Trainium2 Bass kernel for nn_LoraAttention.

Math (reference): qkv = x@W_qkv.T; lora full proj ql/vl = split(x@W_lora.T + b_lora)
(K-part discarded); low-rank dq = (x@A_q.T)@B_q.T*1/8 (same for v); softmax
attention over H=16 heads, D=64; out = attn_cat@W_out.T + b_out.

Host-side algebra folds every LoRA term into the projection weights:
  Wq_eff = W_qkv[q] + W_lora[q] + (B_q@A_q)/8      (q bias b_lora[q] kept)
  Wk_eff = W_qkv[k]                                 (no bias)
  Wv_eff = W_qkv[v] + W_lora[v] + (B_v@A_v)/8
  v bias b_lora[v] commutes through softmax -> folded into host-side output
  bias: b_eff = b_out + W_out @ b_lora[v].

Sharding: 8 cores = 4 batches x 2 head-groups (8 heads each).  Each core
projects QKV for its heads, does attention, and computes a partial output
projection over its 512 concat dims; host sums the two partials per batch.

Device dataflow per core (matmuls bf16 in / fp32 accum), fully pipelined so
the ScalarE exp stream (the bottleneck: 33.5M exps at 1 elem/lane/cycle)
starts as early as possible and never stalls:
  - K/Q projections for head pair t are emitted right before pair t's
    attention; V projection up front (PV needs all token chunks).
  - S^T = K^T@Q per head via row-packed (tile_position) pairs of K=64
    matmuls; exp on ScalarE from PSUM (scale=1/8, bf16 out); P@[V|1] matmuls
    put raw attention in rows 0..63 and the softmax denominator in row 64.
  - normalization inline per (pair, nq): denominator row -> SBUF -> small
    DMA to partition 0, reciprocal (DVE), K=1 ones-matmul broadcast across
    64 partitions, DVE multiply, DMA-pack into (d=128, nq=512) tiles.
  - output projection at the end.
"""

import numpy as np
import ml_dtypes

import concourse.bacc as bacc
import concourse.tile as tile
from concourse import mybir
from concourse.bass_utils import run_bass_kernel_spmd

B, N, C = 4, 2048, 1024
H, D = 16, 64
LORA_SCALE = 1.0 / 8.0
ATTN_SCALE = float(D) ** -0.5  # 0.125

f32 = mybir.dt.float32
bf16 = mybir.dt.bfloat16
BF = ml_dtypes.bfloat16

NQ = 4           # token chunks of 512 for moving operands
MQ = 16          # key/token chunks of 128 for S^T partition dim
KC = 8           # contraction chunks of 128 over C
PAIRS = 4        # head pairs per core (8 local heads)

_cache: dict = {}


def _build_program():
    nc = bacc.Bacc("TRN2", target_bir_lowering=False, debug=False, num_devices=8)

    xT_d = nc.dram_tensor("xT", [C, N], bf16, kind="ExternalInput").ap()
    wqk_d = nc.dram_tensor("wqk", [C, 1024], bf16, kind="ExternalInput").ap()
    wv_d = nc.dram_tensor("wv", [C, 512], bf16, kind="ExternalInput").ap()
    wo_d = nc.dram_tensor("wo", [512, C], bf16, kind="ExternalInput").ap()
    bq_d = nc.dram_tensor("bq", [128, 4], f32, kind="ExternalInput").ap()
    outT_d = nc.dram_tensor("outT", [C, N], f32, kind="ExternalOutput").ap()

    EXP = mybir.ActivationFunctionType.Exp

    with tile.TileContext(nc) as tc:
        with (
            tc.tile_pool(name="win", bufs=1) as win,        # weights + x + consts
            tc.tile_pool(name="kq", bufs=1) as kqp,         # K/Q bf16 tiles
            tc.tile_pool(name="vp", bufs=1) as vp,          # [V|1] tiles
            tc.tile_pool(name="pex", bufs=4) as pex,        # exp outputs
            tc.tile_pool(name="acat", bufs=1) as acatp,     # normalized attn (d, nq)
            tc.tile_pool(name="scr", bufs=4) as scr,        # small scratch
            tc.tile_pool(name="osb", bufs=3) as osbp,       # out eviction
            tc.tile_pool(name="pp", bufs=2, space="PSUM") as pp,
            tc.tile_pool(name="sp", bufs=2, space="PSUM") as spp,
            tc.tile_pool(name="ap", bufs=1, space="PSUM") as app,
        ):
            # ---- loads ----
            xt = []
            for kc in range(KC):
                t = win.tile([128, N], bf16, tag=f"xt{kc}")
                nc.sync.dma_start(t[:], xT_d[kc * 128:(kc + 1) * 128, :])
                xt.append(t)
            wv = []
            for kc in range(KC):
                t = win.tile([128, 512], bf16, tag=f"wv{kc}")
                nc.sync.dma_start(t[:], wv_d[kc * 128:(kc + 1) * 128, :])
                wv.append(t)
            wqk = []
            for kc in range(KC):
                t = win.tile([128, 1024], bf16, tag=f"wqk{kc}")
                nc.sync.dma_start(t[:], wqk_d[kc * 128:(kc + 1) * 128, :])
                wqk.append(t)
            wo = []
            for dc in range(4):
                t = win.tile([128, 1024], bf16, tag=f"wo{dc}")
                nc.sync.dma_start(t[:], wo_d[dc * 128:(dc + 1) * 128, :])
                wo.append(t)
            bqt = win.tile([128, 4], f32, tag="bq")
            nc.sync.dma_start(bqt[:], bq_d[:])
            ones64 = win.tile([1, 64], f32, tag="ones64")
            nc.vector.memset(ones64[:], 1.0)

            acat = [[None] * PAIRS for _ in range(NQ)]

            # ---- V projection: vts[mq] = [x.T chunk].T @ Wv.T -> (token, d) --
            vts = []
            for mq in range(MQ):
                vt = vp.tile([128, 8, 65], bf16, tag=f"v{mq}")
                nc.vector.memset(vt[:, :, 64:65], 1.0)
                ps = pp.tile([128, 512], f32, tag="pp")
                for kc in range(KC):
                    nc.tensor.matmul(
                        ps[:], xt[kc][:, mq * 128:(mq + 1) * 128], wv[kc][:],
                        start=(kc == 0), stop=(kc == KC - 1),
                    )
                nc.vector.tensor_copy(
                    vt[:, :, 0:64], ps[:].rearrange("p (h e) -> p h e", h=8)
                )
                vts.append(vt)

            # ---- per pair: K/Q projection then attention + inline normalize --
            for t in range(PAIRS):
                kt = kqp.tile([128, N], bf16, tag=f"k{t}")
                for nq in range(NQ):
                    ps = pp.tile([128, 512], f32, tag="pp")
                    for kc in range(KC):
                        nc.tensor.matmul(
                            ps[:],
                            wqk[kc][:, 512 + t * 128:512 + (t + 1) * 128],
                            xt[kc][:, nq * 512:(nq + 1) * 512],
                            start=(kc == 0), stop=(kc == KC - 1),
                        )
                    nc.vector.tensor_copy(kt[:, nq * 512:(nq + 1) * 512], ps[:])
                qt = kqp.tile([128, N], bf16, tag=f"q{t}")
                for nq in range(NQ):
                    ps = pp.tile([128, 512], f32, tag="pp")
                    for kc in range(KC):
                        nc.tensor.matmul(
                            ps[:],
                            wqk[kc][:, t * 128:(t + 1) * 128],
                            xt[kc][:, nq * 512:(nq + 1) * 512],
                            start=(kc == 0), stop=(kc == KC - 1),
                        )
                    nc.vector.tensor_scalar_add(
                        qt[:, nq * 512:(nq + 1) * 512], ps[:], bqt[:, t:t + 1]
                    )

                for nq in range(NQ):
                    atA = app.tile([65, 512], f32, tag="atA")
                    atB = app.tile([65, 512], f32, tag="atB")
                    for mq in range(MQ):
                        sp = spp.tile([128, 1024], f32, tag="sp")
                        nc.tensor.matmul(
                            sp[:, 0:512],
                            kt[0:64, mq * 128:(mq + 1) * 128],
                            qt[0:64, nq * 512:(nq + 1) * 512],
                            start=True, stop=True, tile_position=(0, 0),
                        )
                        nc.tensor.matmul(
                            sp[:, 512:1024],
                            kt[64:128, mq * 128:(mq + 1) * 128],
                            qt[64:128, nq * 512:(nq + 1) * 512],
                            start=True, stop=True, tile_position=(64, 0),
                        )
                        pe = pex.tile([128, 1024], bf16, tag="pe")
                        nc.scalar.activation(pe[:], sp[:], EXP, scale=ATTN_SCALE)
                        nc.tensor.matmul(
                            atA[:], vts[mq][:, 2 * t, :], pe[:, 0:512],
                            start=(mq == 0), stop=(mq == MQ - 1),
                        )
                        nc.tensor.matmul(
                            atB[:], vts[mq][:, 2 * t + 1, :], pe[:, 512:1024],
                            start=(mq == 0), stop=(mq == MQ - 1),
                        )
                    # inline normalization for both heads of the pair
                    ac = acatp.tile([128, 512], bf16, tag=f"ac{nq}_{t}")
                    acat[nq][t] = ac
                    for at, half in ((atA, 0), (atB, 1)):
                        ell = scr.tile([65, 512], f32, tag="ell")
                        nc.vector.tensor_copy(ell[64:65, :], at[64:65, :])
                        r0 = scr.tile([1, 512], f32, tag="r0")
                        nc.sync.dma_start(r0[0:1, :], ell[64:65, :])
                        rr = scr.tile([1, 512], f32, tag="rr")
                        nc.vector.reciprocal_approx_fast(rr[0:1, :], r0[0:1, :])
                        rb = pp.tile([64, 512], f32, tag="pp")
                        nc.tensor.matmul(
                            rb[:], ones64[0:1, :], rr[0:1, :],
                            start=True, stop=True,
                        )
                        ar = scr.tile([64, 512], bf16, tag="ar")
                        nc.vector.tensor_copy(ar[:], at[0:64, :])
                        acn = scr.tile([64, 512], bf16, tag="acn")
                        nc.vector.tensor_mul(acn[:], ar[:], rb[:])
                        nc.sync.dma_start(
                            ac[half * 64:(half + 1) * 64, :], acn[:]
                        )

            # ---- output projection: outT[cc, nq] = sum_dc wo[dc].T @ acat ----
            for nq in range(NQ):
                for cc in range(8):
                    ps = pp.tile([128, 512], f32, tag="pp")
                    for dc in range(4):
                        nc.tensor.matmul(
                            ps[:],
                            wo[dc][:, cc * 128:(cc + 1) * 128],
                            acat[nq][dc][:],
                            start=(dc == 0), stop=(dc == 3),
                        )
                    ob = osbp.tile([128, 512], f32, tag="ob")
                    nc.vector.tensor_copy(ob[:], ps[:])
                    nc.sync.dma_start(
                        outT_d[cc * 128:(cc + 1) * 128, nq * 512:(nq + 1) * 512],
                        ob[:],
                    )

    nc.compile()
    return nc


def _get_program():
    if "nc" not in _cache:
        _cache["nc"] = _build_program()
    return _cache["nc"]


def _prep_in_maps(x, W_qkv, W_lora, b_lora, A_q, B_q, A_v, B_v, W_out):
    HD = H * D  # 1024
    Wq = W_qkv[0:HD] + W_lora[0:HD] + LORA_SCALE * (B_q @ A_q)
    Wk = W_qkv[HD:2 * HD]
    Wv = W_qkv[2 * HD:3 * HD] + W_lora[2 * HD:3 * HD] + LORA_SCALE * (B_v @ A_v)
    bq = b_lora[0:HD]

    xT = [np.ascontiguousarray(x[b].T).astype(BF) for b in range(B)]
    in_maps = []
    for c in range(8):
        b, hg = divmod(c, 2)
        sel = slice(hg * 512, (hg + 1) * 512)
        wqk_c = np.ascontiguousarray(
            np.concatenate([Wq[sel], Wk[sel]], axis=0).T
        ).astype(BF)
        wv_c = np.ascontiguousarray(Wv[sel].T).astype(BF)
        wo_c = np.ascontiguousarray(W_out[:, sel].T).astype(BF)
        bq_c = np.ascontiguousarray(bq[sel].reshape(4, 128).T).astype(np.float32)
        in_maps.append({
            "xT": xT[b], "wqk": wqk_c, "wv": wv_c, "wo": wo_c, "bq": bq_c,
        })
    return in_maps


def kernel(x, W_qkv, W_lora, b_lora, A_q, B_q, A_v, B_v, W_out, b_out):
    x = np.asarray(x, np.float32)
    W_qkv = np.asarray(W_qkv, np.float32)
    W_lora = np.asarray(W_lora, np.float32)
    b_lora = np.asarray(b_lora, np.float32)
    A_q = np.asarray(A_q, np.float32)
    B_q = np.asarray(B_q, np.float32)
    A_v = np.asarray(A_v, np.float32)
    B_v = np.asarray(B_v, np.float32)
    W_out = np.asarray(W_out, np.float32)
    b_out = np.asarray(b_out, np.float32)

    in_maps = _prep_in_maps(x, W_qkv, W_lora, b_lora, A_q, B_q, A_v, B_v, W_out)
    b_eff = b_out + W_out @ b_lora[2 * H * D:3 * H * D]

    nc = _get_program()
    res = run_bass_kernel_spmd(nc, in_maps, list(range(8)))

    out = np.empty((B, N, C), np.float32)
    for b in range(B):
        acc = res.results[2 * b]["outT"] + res.results[2 * b + 1]["outT"]
        acc += b_eff[:, None]
        out[b] = acc.T
    return out


# revision 13
# speedup vs baseline: 1.1379x; 1.1158x over previous
"""Trainium2 Bass kernel for nn_LoraAttention.

Math (reference): qkv = x@W_qkv.T; lora full proj ql/vl = split(x@W_lora.T + b_lora)
(K-part discarded); low-rank dq = (x@A_q.T)@B_q.T*1/8 (same for v); softmax
attention over H=16 heads, D=64; out = attn_cat@W_out.T + b_out.

Host-side algebra folds every LoRA term into the projection weights:
  Wq_eff = W_qkv[q] + W_lora[q] + (B_q@A_q)/8      (q bias b_lora[q] kept)
  Wk_eff = W_qkv[k]                                 (no bias)
  Wv_eff = W_qkv[v] + W_lora[v] + (B_v@A_v)/8
  v bias b_lora[v] commutes through softmax -> folded into host-side output
  bias: b_eff = b_out + W_out @ b_lora[v].

Sharding: 8 cores = 4 batches x 2 head-groups (8 heads each).  Each core
projects QKV for its heads, does attention, and computes a partial output
projection over its 512 concat dims; host sums the two partials per batch.

Device dataflow per core (matmuls bf16 in / fp32 accum), fully pipelined so
the ScalarE exp stream (the bottleneck: 33.5M exps at 1 elem/lane/cycle)
starts as early as possible and never stalls:
  - K/Q projections for head pair t are emitted right before pair t's
    attention; V projection up front (PV needs all token chunks).
  - S^T = K^T@Q per head via row-packed (tile_position) pairs of K=64
    matmuls; exp on ScalarE from PSUM (scale=1/8, bf16 out); P@[V|1] matmuls
    put raw attention in rows 0..63 and the softmax denominator in row 64.
  - normalization inline per (pair, nq): denominator row -> SBUF -> small
    DMA to partition 0, reciprocal (DVE), K=1 ones-matmul broadcast across
    64 partitions, DVE multiply, DMA-pack into (d=128, nq=512) tiles.
  - output projection at the end.
"""

import numpy as np
import ml_dtypes

import concourse.bacc as bacc
import concourse.tile as tile
from concourse import mybir
from concourse.bass_utils import run_bass_kernel_spmd

B, N, C = 4, 2048, 1024
H, D = 16, 64
LORA_SCALE = 1.0 / 8.0
ATTN_SCALE = float(D) ** -0.5  # 0.125

f32 = mybir.dt.float32
bf16 = mybir.dt.bfloat16
BF = ml_dtypes.bfloat16

NQ = 4           # token chunks of 512 for moving operands
MQ = 16          # key/token chunks of 128 for S^T partition dim
KC = 8           # contraction chunks of 128 over C
PAIRS = 4        # head pairs per core (8 local heads)

_cache: dict = {}


def _build_program():
    nc = bacc.Bacc("TRN2", target_bir_lowering=False, debug=False, num_devices=8)

    xT_d = nc.dram_tensor("xT", [C, N], bf16, kind="ExternalInput").ap()
    wqk_d = nc.dram_tensor("wqk", [C, 1024], bf16, kind="ExternalInput").ap()
    wv_d = nc.dram_tensor("wv", [C, 512], bf16, kind="ExternalInput").ap()
    wo_d = nc.dram_tensor("wo", [512, C], bf16, kind="ExternalInput").ap()
    bq_d = nc.dram_tensor("bq", [128, 4], f32, kind="ExternalInput").ap()
    outT_d = nc.dram_tensor("outT", [C, N], f32, kind="ExternalOutput").ap()

    EXP = mybir.ActivationFunctionType.Exp

    with tile.TileContext(nc) as tc:
        with (
            tc.tile_pool(name="win", bufs=1) as win,        # weights + x + consts
            tc.tile_pool(name="kq", bufs=1) as kqp,         # K/Q bf16 tiles
            tc.tile_pool(name="vp", bufs=1) as vp,          # [V|1] tiles
            tc.tile_pool(name="pex", bufs=6) as pex,        # exp outputs
            tc.tile_pool(name="acat", bufs=1) as acatp,     # normalized attn (d, nq)
            tc.tile_pool(name="scr", bufs=4) as scr,        # small scratch
            tc.tile_pool(name="osb", bufs=3) as osbp,       # out eviction
            tc.tile_pool(name="pp", bufs=2, space="PSUM") as pp,
            tc.tile_pool(name="sp", bufs=2, space="PSUM") as spp,
            tc.tile_pool(name="ap", bufs=1, space="PSUM") as app,
        ):
            # ---- loads ----
            xt = []
            for kc in range(KC):
                t = win.tile([128, N], bf16, tag=f"xt{kc}")
                nc.sync.dma_start(t[:], xT_d[kc * 128:(kc + 1) * 128, :])
                xt.append(t)
            wqk = []
            for kc in range(KC):
                t = win.tile([128, 1024], bf16, tag=f"wqk{kc}")
                nc.sync.dma_start(t[:], wqk_d[kc * 128:(kc + 1) * 128, :])
                wqk.append(t)
            wv = []
            for kc in range(KC):
                t = win.tile([128, 512], bf16, tag=f"wv{kc}")
                nc.sync.dma_start(t[:], wv_d[kc * 128:(kc + 1) * 128, :])
                wv.append(t)
            wo = []
            for dc in range(4):
                t = win.tile([128, 1024], bf16, tag=f"wo{dc}")
                nc.sync.dma_start(t[:], wo_d[dc * 128:(dc + 1) * 128, :])
                wo.append(t)
            bqt = win.tile([128, 4], f32, tag="bq")
            nc.sync.dma_start(bqt[:], bq_d[:])
            ones64 = win.tile([1, 64], f32, tag="ones64")
            nc.vector.memset(ones64[:], 1.0)

            acat = [[None] * PAIRS for _ in range(NQ)]

            def kq_proj(t):
                kt = kqp.tile([128, N], bf16, tag=f"k{t}")
                for nq in range(NQ):
                    ps = pp.tile([128, 512], f32, tag="pp")
                    for kc in range(KC):
                        nc.tensor.matmul(
                            ps[:],
                            wqk[kc][:, 512 + t * 128:512 + (t + 1) * 128],
                            xt[kc][:, nq * 512:(nq + 1) * 512],
                            start=(kc == 0), stop=(kc == KC - 1),
                        )
                    nc.vector.tensor_copy(kt[:, nq * 512:(nq + 1) * 512], ps[:])
                qt = kqp.tile([128, N], bf16, tag=f"q{t}")
                for nq in range(NQ):
                    ps = pp.tile([128, 512], f32, tag="pp")
                    for kc in range(KC):
                        nc.tensor.matmul(
                            ps[:],
                            wqk[kc][:, t * 128:(t + 1) * 128],
                            xt[kc][:, nq * 512:(nq + 1) * 512],
                            start=(kc == 0), stop=(kc == KC - 1),
                        )
                    nc.vector.tensor_scalar_add(
                        qt[:, nq * 512:(nq + 1) * 512], ps[:], bqt[:, t:t + 1]
                    )
                return kt, qt

            kq_tiles = {0: kq_proj(0)}

            # ---- V projection: vts[mq] = [x.T chunk].T @ Wv.T -> (token, d) --
            vts = []
            for mq in range(MQ):
                vt = vp.tile([128, 8, 65], bf16, tag=f"v{mq}")
                nc.vector.memset(vt[:, :, 64:65], 1.0)
                ps = pp.tile([128, 512], f32, tag="pp")
                for kc in range(KC):
                    nc.tensor.matmul(
                        ps[:], xt[kc][:, mq * 128:(mq + 1) * 128], wv[kc][:],
                        start=(kc == 0), stop=(kc == KC - 1),
                    )
                nc.vector.tensor_copy(
                    vt[:, :, 0:64], ps[:].rearrange("p (h e) -> p h e", h=8)
                )
                vts.append(vt)

            # ---- per pair: attention + inline normalize; prefetch next proj --
            for t in range(PAIRS):
                kt, qt = kq_tiles.pop(t)
                for nq in range(NQ):
                    atA = app.tile([65, 512], f32, tag="atA")
                    atB = app.tile([65, 512], f32, tag="atB")
                    for mq in range(MQ):
                        sp = spp.tile([128, 1024], f32, tag="sp")
                        nc.tensor.matmul(
                            sp[:, 0:512],
                            kt[0:64, mq * 128:(mq + 1) * 128],
                            qt[0:64, nq * 512:(nq + 1) * 512],
                            start=True, stop=True, tile_position=(0, 0),
                        )
                        nc.tensor.matmul(
                            sp[:, 512:1024],
                            kt[64:128, mq * 128:(mq + 1) * 128],
                            qt[64:128, nq * 512:(nq + 1) * 512],
                            start=True, stop=True, tile_position=(64, 0),
                        )
                        pe = pex.tile([128, 1024], bf16, tag="pe")
                        nc.scalar.activation(pe[:], sp[:], EXP, scale=ATTN_SCALE)
                        nc.tensor.matmul(
                            atA[:], vts[mq][:, 2 * t, :], pe[:, 0:512],
                            start=(mq == 0), stop=(mq == MQ - 1),
                        )
                        nc.tensor.matmul(
                            atB[:], vts[mq][:, 2 * t + 1, :], pe[:, 512:1024],
                            start=(mq == 0), stop=(mq == MQ - 1),
                        )
                    # inline normalization for both heads of the pair
                    ac = acatp.tile([128, 512], bf16, tag=f"ac{nq}_{t}")
                    acat[nq][t] = ac
                    for at, half in ((atA, 0), (atB, 1)):
                        ell = scr.tile([65, 512], f32, tag="ell")
                        nc.vector.tensor_copy(ell[64:65, :], at[64:65, :])
                        r0 = scr.tile([1, 512], f32, tag="r0")
                        nc.sync.dma_start(r0[0:1, :], ell[64:65, :])
                        rr = scr.tile([1, 512], f32, tag="rr")
                        nc.vector.reciprocal_approx_fast(rr[0:1, :], r0[0:1, :])
                        rb = app.tile([64, 512], f32, tag="atA")
                        nc.tensor.matmul(
                            rb[:], ones64[0:1, :], rr[0:1, :],
                            start=True, stop=True,
                        )
                        ar = scr.tile([64, 512], bf16, tag="ar")
                        nc.vector.tensor_copy(ar[:], at[0:64, :])
                        acn = scr.tile([64, 512], bf16, tag="acn")
                        nc.vector.tensor_mul(acn[:], ar[:], rb[:])
                        nc.sync.dma_start(
                            ac[half * 64:(half + 1) * 64, :], acn[:]
                        )
                    if nq == 0 and t + 1 < PAIRS:
                        kq_tiles[t + 1] = kq_proj(t + 1)

            # ---- output projection: outT[cc, nq] = sum_dc wo[dc].T @ acat ----
            for nq in range(NQ):
                for cc in range(8):
                    ps = pp.tile([128, 512], f32, tag="pp")
                    for dc in range(4):
                        nc.tensor.matmul(
                            ps[:],
                            wo[dc][:, cc * 128:(cc + 1) * 128],
                            acat[nq][dc][:],
                            start=(dc == 0), stop=(dc == 3),
                        )
                    ob = osbp.tile([128, 512], f32, tag="ob")
                    nc.vector.tensor_copy(ob[:], ps[:])
                    nc.sync.dma_start(
                        outT_d[cc * 128:(cc + 1) * 128, nq * 512:(nq + 1) * 512],
                        ob[:],
                    )

    nc.compile()
    return nc


def _get_program():
    if "nc" not in _cache:
        _cache["nc"] = _build_program()
    return _cache["nc"]


def _prep_in_maps(x, W_qkv, W_lora, b_lora, A_q, B_q, A_v, B_v, W_out):
    HD = H * D  # 1024
    Wq = W_qkv[0:HD] + W_lora[0:HD] + LORA_SCALE * (B_q @ A_q)
    Wk = W_qkv[HD:2 * HD]
    Wv = W_qkv[2 * HD:3 * HD] + W_lora[2 * HD:3 * HD] + LORA_SCALE * (B_v @ A_v)
    bq = b_lora[0:HD]

    xT = [np.ascontiguousarray(x[b].T).astype(BF) for b in range(B)]
    in_maps = []
    for c in range(8):
        b, hg = divmod(c, 2)
        sel = slice(hg * 512, (hg + 1) * 512)
        wqk_c = np.ascontiguousarray(
            np.concatenate([Wq[sel], Wk[sel]], axis=0).T
        ).astype(BF)
        wv_c = np.ascontiguousarray(Wv[sel].T).astype(BF)
        wo_c = np.ascontiguousarray(W_out[:, sel].T).astype(BF)
        bq_c = np.ascontiguousarray(bq[sel].reshape(4, 128).T).astype(np.float32)
        in_maps.append({
            "xT": xT[b], "wqk": wqk_c, "wv": wv_c, "wo": wo_c, "bq": bq_c,
        })
    return in_maps


def kernel(x, W_qkv, W_lora, b_lora, A_q, B_q, A_v, B_v, W_out, b_out):
    x = np.asarray(x, np.float32)
    W_qkv = np.asarray(W_qkv, np.float32)
    W_lora = np.asarray(W_lora, np.float32)
    b_lora = np.asarray(b_lora, np.float32)
    A_q = np.asarray(A_q, np.float32)
    B_q = np.asarray(B_q, np.float32)
    A_v = np.asarray(A_v, np.float32)
    B_v = np.asarray(B_v, np.float32)
    W_out = np.asarray(W_out, np.float32)
    b_out = np.asarray(b_out, np.float32)

    in_maps = _prep_in_maps(x, W_qkv, W_lora, b_lora, A_q, B_q, A_v, B_v, W_out)
    b_eff = b_out + W_out @ b_lora[2 * H * D:3 * H * D]

    nc = _get_program()
    res = run_bass_kernel_spmd(nc, in_maps, list(range(8)))

    out = np.empty((B, N, C), np.float32)
    for b in range(B):
        acc = res.results[2 * b]["outT"] + res.results[2 * b + 1]["outT"]
        acc += b_eff[:, None]
        out[b] = acc.T
    return out
